# revision 2
# baseline (speedup 1.0000x reference)
"""Trainium2 Bass kernel for the compositional skeleton loss.

loss = mean_b sum_{pairs p, xyz c} | (C @ bones_in)[b,p,c] - (T @ bones_tgt)[b,p,c] |

Reformulated as one matmul per batch row:  delta_row = z_row @ W, where
z_row = [input_row (63), target_row (63)] and W is [126, 630] built from the
signed path-sum matrix C and the endpoint-diff matrix T (block structure over
the 3 xyz channels), followed by abs + total sum, / B.

v2: float32r matmuls + transposes (1 cycle/row vs 4 for fp32 on the PE),
contiguous-per-partition DMA staging, greedy ACT/DVE balance of the
PSUM abs+accumulate drain.

Sharding: pure data parallel over the batch dim across 8 NeuronCores.
Each core returns per-partition partial sums [126,1]; host adds them up.
"""

import numpy as np
from collections import deque
from itertools import combinations

# ---------------------------------------------------------------- constants
NJ = 21
B_FULL = 65536
N_CORES = 8
B_CORE = B_FULL // N_CORES  # 8192

_JOINTS = ['Ab', 'Chest', 'Head', 'Hip', 'LFArm', 'LFoot', 'LHand', 'LShin',
           'LShoulder', 'LThigh', 'LToe', 'LUArm', 'Neck', 'RFArm', 'RFoot',
           'RHand', 'RShin', 'RShoulder', 'RThigh', 'RToe', 'RUArm']
_PARENTS = {'Ab': 'Hip', 'Chest': 'Ab', 'Head': 'Neck', 'Hip': 'Hip',
            'LFArm': 'LUArm', 'LFoot': 'LShin', 'LHand': 'LFArm',
            'LShin': 'LThigh', 'LShoulder': 'Chest', 'LThigh': 'Hip',
            'LToe': 'LFoot', 'LUArm': 'LShoulder', 'Neck': 'Chest',
            'RFArm': 'RUArm', 'RFoot': 'RShin', 'RHand': 'RFArm',
            'RShin': 'RThigh', 'RShoulder': 'Chest', 'RThigh': 'Hip',
            'RToe': 'RFoot', 'RUArm': 'RShoulder'}


def _build_w():
    idx = {n: i for i, n in enumerate(_JOINTS)}
    par = {idx[k]: idx[v] for k, v in _PARENTS.items()}
    adj = {j: [] for j in range(NJ)}
    for j, p in par.items():
        if j != p:
            adj[j].append(p)
            adj[p].append(j)

    def bfs_path(u, v):
        prev = {u: None}
        q = deque([u])
        while q:
            x = q.popleft()
            if x == v:
                break
            for y in adj[x]:
                if y not in prev:
                    prev[y] = x
                    q.append(y)
        path = [v]
        while prev[path[-1]] is not None:
            path.append(prev[path[-1]])
        return path[::-1]

    pairs = list(combinations(range(NJ), 2))  # 210
    c_np = np.zeros((len(pairs), NJ), np.float32)
    t_np = np.zeros((len(pairs), NJ), np.float32)
    for pi, (u, v) in enumerate(pairs):
        pa = bfs_path(u, v)
        for m in range(len(pa) - 1):
            c_np[pi, pa[m]] += 1.0 if par[pa[m]] == pa[m + 1] else -1.0
        t_np[pi, u] += 1.0
        t_np[pi, v] -= 1.0

    # W[t*63 + j*3 + c, p*3 + c] = C[p,j] (t=0) / -T[p,j] (t=1)
    eye3 = np.eye(3, dtype=np.float32)
    w_in = np.einsum('pj,cd->jcpd', c_np, eye3).reshape(63, 630)
    w_tg = np.einsum('pj,cd->jcpd', -t_np, eye3).reshape(63, 630)
    return np.ascontiguousarray(np.concatenate([w_in, w_tg], axis=0))  # [126, 630]


_W = _build_w()

# ---------------------------------------------------------------- bass build
R_PER_GRP = 8                       # 128-row tiles per group
N_GRP = B_CORE // (128 * R_PER_GRP)  # 8
N_CCH = 5                           # 630 = 5 x 126 output-column chunks

_NC = None


def _build_bass(n_reps=1):
    import concourse.bacc as bacc
    import concourse.mybir as mybir
    import concourse.tile as tile

    f32 = mybir.dt.float32
    f32r = mybir.dt.float32r
    nc = bacc.Bacc("TRN2", target_bir_lowering=False, debug=False)

    x = nc.dram_tensor("x", [B_CORE, 63], f32, kind="ExternalInput")
    y = nc.dram_tensor("y", [B_CORE, 63], f32, kind="ExternalInput")
    out = nc.dram_tensor("out", [126, 1], f32, kind="ExternalOutput")

    w_dram = nc.inline_tensor(_W, name="w_const")
    ident_dram = nc.inline_tensor(np.eye(128, dtype=np.float32), name="ident_const")

    with tile.TileContext(nc) as tc:
        with (
            tc.tile_pool(name="consts", bufs=1) as consts,
            tc.tile_pool(name="staged", bufs=3) as staged_pool,
            tc.tile_pool(name="zt", bufs=3) as zt_pool,
            tc.tile_pool(name="accp", bufs=2) as acc_pool,
            tc.tile_pool(name="psumT", bufs=1, space="PSUM") as psumT_pool,
            tc.tile_pool(name="psumD", bufs=3, space="PSUM") as psumD_pool,
            tc.tile_pool(name="misc", bufs=1) as misc,
        ):
            w_sb = consts.tile([126, 630], f32)
            nc.sync.dma_start(w_sb[:], w_dram[:])
            id_sb = consts.tile([128, 128], f32)
            nc.sync.dma_start(id_sb[:], ident_dram[:])

            scratch = misc.tile([126, 1024], f32)  # ACT abs dump (never read)

            # greedy ACT/DVE balance (errata-adjusted ns-per-op estimates:
            # ACT psum (172+FD)/1.2, DVE psum (120+FD)/0.96 at FD=1024)
            eng_t = {"act": 0.0, "dve": 0.0}

            def pick_engine(act_ns, dve_ns):
                e = "act" if eng_t["act"] + act_ns <= \
                    eng_t["dve"] + dve_ns else "dve"
                eng_t[e] += act_ns if e == "act" else dve_ns
                return e

            def emit_copy(dst, src):
                if pick_engine(997.0, 1192.0) == "act":
                    nc.scalar.copy(dst, src)
                else:
                    nc.vector.tensor_copy(dst, src)

            def emit_absred(col, dps):
                if pick_engine(997.0, 1192.0) == "act":
                    nc.scalar.activation(
                        scratch[:], dps[:],
                        mybir.ActivationFunctionType.Abs, accum_out=col)
                else:
                    nc.vector.tensor_reduce(
                        col, dps[:], axis=mybir.AxisListType.X,
                        op=mybir.AluOpType.add, apply_absolute_value=True)

            rows = 128 * R_PER_GRP  # 1024

            for rep in range(n_reps):
                acc = acc_pool.tile([126, N_GRP * N_CCH], f32, tag="acc")
                final = acc_pool.tile([126, 1], f32, tag="final")

                prev = None  # (zt, g) pending matmul+epilogue

                def flush_prev():
                    zt, g = prev
                    zr = zt[:].bitcast(f32r)
                    for c in range(N_CCH):
                        # two fp32r matmuls fill a 2-bank psum tile; one
                        # 1024-wide fused abs+sum drains it
                        dps = psumD_pool.tile([126, 1024], f32)
                        wc = w_sb[:, c * 126:(c + 1) * 126].bitcast(f32r)
                        nc.tensor.matmul(
                            dps[:, 0:512], wc, zr[:, 0:512])
                        nc.tensor.matmul(
                            dps[:, 512:1024], wc, zr[:, 512:1024])
                        emit_absred(
                            acc[:, g * N_CCH + c: g * N_CCH + c + 1], dps)

                for g in range(N_GRP):
                    # staging: x rows -> [:, 0, :, :], y rows -> [:, 1, :, :]
                    # so each partition receives one contiguous 2016B run per
                    # tensor (8 consecutive DRAM rows of 252B)
                    st = staged_pool.tile([128, 2, R_PER_GRP, 63], f32)
                    xv = x[g * rows:(g + 1) * rows, :].rearrange(
                        "(p r) j -> p (r j)", p=128)
                    yv = y[g * rows:(g + 1) * rows, :].rearrange(
                        "(p r) j -> p (r j)", p=128)
                    nc.sync.dma_start(st[:, 0, :, :], xv)
                    nc.sync.dma_start(st[:, 1, :, :], yv)

                    # transpose 8x [128,126] (x block + y block via 2-level
                    # free AP) -> one [126,1024] psum tile, fp32r streaming
                    zt_ps = psumT_pool.tile([126, 1024], f32)
                    for r in range(R_PER_GRP):
                        nc.tensor.transpose(
                            zt_ps[:, r * 128:(r + 1) * 128].bitcast(f32r),
                            st[:, :, r, :].bitcast(f32r),
                            id_sb[:].bitcast(f32r))

                    zt = zt_pool.tile([126, 1024], f32)
                    emit_copy(zt[:], zt_ps[:])

                    # software pipeline: matmuls for the PREVIOUS group run
                    # after this group's transposes, so the PE never waits
                    # on the psum->sbuf copy of its rhs
                    if prev is not None:
                        flush_prev()
                    prev = (zt, g)

                flush_prev()

                nc.vector.tensor_reduce(
                    final[:], acc[:], axis=mybir.AxisListType.X,
                    op=mybir.AluOpType.add)
                nc.sync.dma_start(out[:], final[:])

    nc.compile()
    return nc


def kernel(input, target):
    global _NC
    from concourse.bass_utils import run_bass_kernel_spmd

    if _NC is None:
        _NC = _build_bass()

    inp = np.ascontiguousarray(np.asarray(input, dtype=np.float32))
    tgt = np.ascontiguousarray(np.asarray(target, dtype=np.float32))
    assert inp.shape == (B_FULL, NJ * 3) and tgt.shape == (B_FULL, NJ * 3)

    in_maps = []
    for i in range(N_CORES):
        sl = slice(i * B_CORE, (i + 1) * B_CORE)
        in_maps.append({
            "x": np.ascontiguousarray(inp[sl]),
            "y": np.ascontiguousarray(tgt[sl]),
        })

    res = run_bass_kernel_spmd(_NC, in_maps, core_ids=list(range(N_CORES)))
    total = np.float64(0.0)
    for r in res.results:
        total += np.float64(r["out"].astype(np.float64).sum())
    return np.array([total / B_FULL], dtype=np.float32)


# revision 7
# speedup vs baseline: 57.8193x; 57.8193x over previous
"""Trainium2 Bass kernel for the compositional skeleton loss.

loss = mean_b sum_{pairs p, xyz c} | (C @ bones_in)[b,p,c] - (T @ bones_tgt)[b,p,c] |

Reformulated as one matmul per batch row:  delta_row = z_row @ W, where
z_row = [input_row (63), target_row (63)] and W is [126, 630] built from the
signed path-sum matrix C and the endpoint-diff matrix T (block structure over
the 3 xyz channels), followed by abs + total sum, / B.

v2: float32r matmuls + transposes (1 cycle/row vs 4 for fp32 on the PE),
contiguous-per-partition DMA staging, greedy ACT/DVE balance of the
PSUM abs+accumulate drain.

Sharding: pure data parallel over the batch dim across 8 NeuronCores.
Each core returns per-partition partial sums [126,1]; host adds them up.
"""

import numpy as np
from collections import deque
from itertools import combinations

# ---------------------------------------------------------------- constants
NJ = 21
B_FULL = 65536
N_CORES = 8
B_CORE = B_FULL // N_CORES  # 8192

_JOINTS = ['Ab', 'Chest', 'Head', 'Hip', 'LFArm', 'LFoot', 'LHand', 'LShin',
           'LShoulder', 'LThigh', 'LToe', 'LUArm', 'Neck', 'RFArm', 'RFoot',
           'RHand', 'RShin', 'RShoulder', 'RThigh', 'RToe', 'RUArm']
_PARENTS = {'Ab': 'Hip', 'Chest': 'Ab', 'Head': 'Neck', 'Hip': 'Hip',
            'LFArm': 'LUArm', 'LFoot': 'LShin', 'LHand': 'LFArm',
            'LShin': 'LThigh', 'LShoulder': 'Chest', 'LThigh': 'Hip',
            'LToe': 'LFoot', 'LUArm': 'LShoulder', 'Neck': 'Chest',
            'RFArm': 'RUArm', 'RFoot': 'RShin', 'RHand': 'RFArm',
            'RShin': 'RThigh', 'RShoulder': 'Chest', 'RThigh': 'Hip',
            'RToe': 'RFoot', 'RUArm': 'RShoulder'}


def _build_w():
    idx = {n: i for i, n in enumerate(_JOINTS)}
    par = {idx[k]: idx[v] for k, v in _PARENTS.items()}
    adj = {j: [] for j in range(NJ)}
    for j, p in par.items():
        if j != p:
            adj[j].append(p)
            adj[p].append(j)

    def bfs_path(u, v):
        prev = {u: None}
        q = deque([u])
        while q:
            x = q.popleft()
            if x == v:
                break
            for y in adj[x]:
                if y not in prev:
                    prev[y] = x
                    q.append(y)
        path = [v]
        while prev[path[-1]] is not None:
            path.append(prev[path[-1]])
        return path[::-1]

    pairs = list(combinations(range(NJ), 2))  # 210
    c_np = np.zeros((len(pairs), NJ), np.float32)
    t_np = np.zeros((len(pairs), NJ), np.float32)
    for pi, (u, v) in enumerate(pairs):
        pa = bfs_path(u, v)
        for m in range(len(pa) - 1):
            c_np[pi, pa[m]] += 1.0 if par[pa[m]] == pa[m + 1] else -1.0
        t_np[pi, u] += 1.0
        t_np[pi, v] -= 1.0

    # W[t*63 + j*3 + c, p*3 + c] = C[p,j] (t=0) / -T[p,j] (t=1)
    eye3 = np.eye(3, dtype=np.float32)
    w_in = np.einsum('pj,cd->jcpd', c_np, eye3).reshape(63, 630)
    w_tg = np.einsum('pj,cd->jcpd', -t_np, eye3).reshape(63, 630)
    return np.ascontiguousarray(np.concatenate([w_in, w_tg], axis=0))  # [126, 630]


_W = _build_w()

# ---------------------------------------------------------------- bass build
R_PER_GRP = 8                       # 128-row tiles per group
N_GRP = B_CORE // (128 * R_PER_GRP)  # 8
N_CCH = 5                           # 630 = 5 x 126 output-column chunks

_NC = None


def _build_bass(n_reps=1):
    import concourse.bacc as bacc
    import concourse.mybir as mybir
    import concourse.tile as tile

    f32 = mybir.dt.float32
    f32r = mybir.dt.float32r
    nc = bacc.Bacc("TRN2", target_bir_lowering=False, debug=False)

    # The whole PE input chain is declared float32r so every producer
    # (DMA, psum->sbuf copy) emits fp32r-typed outputs, which the BIR
    # verifier requires for fp32r matmul inputs.  numpy binding is
    # unchanged (dt.np(float32r) == np.float32); the PE rounds internally.
    x = nc.dram_tensor("x", [B_CORE, 63], f32r, kind="ExternalInput")
    y = nc.dram_tensor("y", [B_CORE, 63], f32r, kind="ExternalInput")
    out = nc.dram_tensor("out", [126, 1], f32, kind="ExternalOutput")

    w_dram = nc.inline_tensor(_W, name="w_const")
    ident_dram = nc.inline_tensor(np.eye(128, dtype=np.float32), name="ident_const")

    with tile.TileContext(nc) as tc:
        with (
            tc.tile_pool(name="consts", bufs=1) as consts,
            tc.tile_pool(name="staged", bufs=3) as staged_pool,
            tc.tile_pool(name="zt", bufs=3) as zt_pool,
            tc.tile_pool(name="accp", bufs=2) as acc_pool,
            tc.tile_pool(name="psumT", bufs=1, space="PSUM") as psumT_pool,
            tc.tile_pool(name="psumD", bufs=3, space="PSUM") as psumD_pool,
            tc.tile_pool(name="misc", bufs=1) as misc,
        ):
            w_sb = consts.tile([126, 630], f32r)
            nc.sync.dma_start(w_sb[:], w_dram[:].bitcast(f32r))
            id_sb = consts.tile([128, 128], f32r)
            nc.sync.dma_start(id_sb[:], ident_dram[:].bitcast(f32r))

            scratch = misc.tile([126, 1024], f32)  # ACT abs dump (never read)

            # greedy ACT/DVE balance (errata-adjusted ns-per-op estimates:
            # ACT psum (172+FD)/1.2, DVE psum (120+FD)/0.96 at FD=1024)
            eng_t = {"act": 0.0, "dve": 0.0}

            def pick_engine(act_ns, dve_ns):
                e = "act" if eng_t["act"] + act_ns <= \
                    eng_t["dve"] + dve_ns else "dve"
                eng_t[e] += act_ns if e == "act" else dve_ns
                return e

            def emit_copy(dst, src):
                if pick_engine(997.0, 1192.0) == "act":
                    nc.scalar.copy(dst, src)
                else:
                    nc.vector.tensor_copy(dst, src)

            def emit_absred(col, dps):
                if pick_engine(997.0, 1192.0) == "act":
                    nc.scalar.activation(
                        scratch[:], dps[:],
                        mybir.ActivationFunctionType.Abs, accum_out=col)
                else:
                    nc.vector.tensor_reduce(
                        col, dps[:], axis=mybir.AxisListType.X,
                        op=mybir.AluOpType.add, apply_absolute_value=True)

            rows = 128 * R_PER_GRP  # 1024

            for rep in range(n_reps):
                acc = acc_pool.tile([126, N_GRP * N_CCH], f32, tag="acc")
                final = acc_pool.tile([126, 1], f32, tag="final")

                prev = None  # (zt, g) pending matmul+epilogue

                def flush_prev():
                    zt, g = prev
                    for c in range(N_CCH):
                        # two fp32r matmuls fill a 2-bank psum tile; one
                        # 1024-wide fused abs+sum drains it
                        dps = psumD_pool.tile([126, 1024], f32)
                        wc = w_sb[:, c * 126:(c + 1) * 126]
                        nc.tensor.matmul(
                            dps[:, 0:512], wc, zt[:, 0:512])
                        nc.tensor.matmul(
                            dps[:, 512:1024], wc, zt[:, 512:1024])
                        emit_absred(
                            acc[:, g * N_CCH + c: g * N_CCH + c + 1], dps)

                for g in range(N_GRP):
                    st = staged_pool.tile([128, R_PER_GRP, 126], f32r)
                    xv = x[g * rows:(g + 1) * rows, :].rearrange(
                        "(p r) j -> p r j", p=128)
                    yv = y[g * rows:(g + 1) * rows, :].rearrange(
                        "(p r) j -> p r j", p=128)
                    nc.sync.dma_start(st[:, :, 0:63], xv)
                    nc.sync.dma_start(st[:, :, 63:126], yv)

                    # transpose 8x [128,126] -> one [126,1024] psum tile,
                    # fp32r streaming (1.5 cycles/row vs 2 for fp32)
                    zt_ps = psumT_pool.tile([126, 1024], f32r)
                    for r in range(R_PER_GRP):
                        nc.tensor.transpose(
                            zt_ps[:, r * 128:(r + 1) * 128],
                            st[:, r, :],
                            id_sb[:])

                    zt = zt_pool.tile([126, 1024], f32r)
                    emit_copy(zt[:], zt_ps[:])

                    # software pipeline: matmuls for the PREVIOUS group run
                    # after this group's transposes, so the PE never waits
                    # on the psum->sbuf copy of its rhs
                    if prev is not None:
                        flush_prev()
                    prev = (zt, g)

                flush_prev()

                nc.vector.tensor_reduce(
                    final[:], acc[:], axis=mybir.AxisListType.X,
                    op=mybir.AluOpType.add)
                nc.sync.dma_start(out[:], final[:])

    nc.compile()
    return nc


def kernel(input, target):
    global _NC
    from concourse.bass_utils import run_bass_kernel_spmd

    if _NC is None:
        _NC = _build_bass()

    inp = np.ascontiguousarray(np.asarray(input, dtype=np.float32))
    tgt = np.ascontiguousarray(np.asarray(target, dtype=np.float32))
    assert inp.shape == (B_FULL, NJ * 3) and tgt.shape == (B_FULL, NJ * 3)

    in_maps = []
    for i in range(N_CORES):
        sl = slice(i * B_CORE, (i + 1) * B_CORE)
        in_maps.append({
            "x": np.ascontiguousarray(inp[sl]),
            "y": np.ascontiguousarray(tgt[sl]),
        })

    res = run_bass_kernel_spmd(_NC, in_maps, core_ids=list(range(N_CORES)))
    total = np.float64(0.0)
    for r in res.results:
        total += np.float64(r["out"].astype(np.float64).sum())
    return np.array([total / B_FULL], dtype=np.float32)


# revision 9
# speedup vs baseline: 569.5135x; 9.8499x over previous
"""Trainium2 Bass kernel for the compositional skeleton loss.

loss = mean_b sum_{pairs p, xyz c} | (C @ bones_in)[b,p,c] - (T @ bones_tgt)[b,p,c] |

Reformulated as one matmul per batch row:  delta_row = z_row @ W, where
z_row = [input_row (63), target_row (63)] and W is [126, 630] built from the
signed path-sum matrix C and the endpoint-diff matrix T (block structure over
the 3 xyz channels), followed by abs + total sum, / B.

v5 pipeline (per 8192-row core shard):
  DMA      x,y -> separate contiguous staging tiles (1 descriptor/partition;
           the old interleaved layout fragmented every DMA into 252B runs and
           made the kernel DMA-descriptor-bound at ~32us)
  GPSIMD   fp32 -> bf16 cast, interleaving x|y into [128, 8, 126] tiles
           (strided engine writes are free; DMA strides are not)
  PE       8x [128,126] transposes per 1024-row group (bf16, 1 cyc/row,
           1-bank psum) + 10 bf16 matmuls vs W chunks (1 cyc/row)
  DVE      psum->sbuf copies of z^T (2x mode on 16-bit)
  ACT/DVE  fused abs+accumulate drains of the [126,1024] fp32 psum tiles,
           greedy-balanced by measured per-op cost
Sharding: pure data parallel over batch across 8 NeuronCores; host sums the
eight [126,1] partial outputs.
"""

import numpy as np
from collections import deque
from itertools import combinations

# ---------------------------------------------------------------- constants
NJ = 21
B_FULL = 65536
N_CORES = 8
B_CORE = B_FULL // N_CORES  # 8192

_JOINTS = ['Ab', 'Chest', 'Head', 'Hip', 'LFArm', 'LFoot', 'LHand', 'LShin',
           'LShoulder', 'LThigh', 'LToe', 'LUArm', 'Neck', 'RFArm', 'RFoot',
           'RHand', 'RShin', 'RShoulder', 'RThigh', 'RToe', 'RUArm']
_PARENTS = {'Ab': 'Hip', 'Chest': 'Ab', 'Head': 'Neck', 'Hip': 'Hip',
            'LFArm': 'LUArm', 'LFoot': 'LShin', 'LHand': 'LFArm',
            'LShin': 'LThigh', 'LShoulder': 'Chest', 'LThigh': 'Hip',
            'LToe': 'LFoot', 'LUArm': 'LShoulder', 'Neck': 'Chest',
            'RFArm': 'RUArm', 'RFoot': 'RShin', 'RHand': 'RFArm',
            'RShin': 'RThigh', 'RShoulder': 'Chest', 'RThigh': 'Hip',
            'RToe': 'RFoot', 'RUArm': 'RShoulder'}


def _build_w():
    idx = {n: i for i, n in enumerate(_JOINTS)}
    par = {idx[k]: idx[v] for k, v in _PARENTS.items()}
    adj = {j: [] for j in range(NJ)}
    for j, p in par.items():
        if j != p:
            adj[j].append(p)
            adj[p].append(j)

    def bfs_path(u, v):
        prev = {u: None}
        q = deque([u])
        while q:
            x = q.popleft()
            if x == v:
                break
            for y in adj[x]:
                if y not in prev:
                    prev[y] = x
                    q.append(y)
        path = [v]
        while prev[path[-1]] is not None:
            path.append(prev[path[-1]])
        return path[::-1]

    pairs = list(combinations(range(NJ), 2))  # 210
    c_np = np.zeros((len(pairs), NJ), np.float32)
    t_np = np.zeros((len(pairs), NJ), np.float32)
    for pi, (u, v) in enumerate(pairs):
        pa = bfs_path(u, v)
        for m in range(len(pa) - 1):
            c_np[pi, pa[m]] += 1.0 if par[pa[m]] == pa[m + 1] else -1.0
        t_np[pi, u] += 1.0
        t_np[pi, v] -= 1.0

    # W[t*63 + j*3 + c, p*3 + c] = C[p,j] (t=0) / -T[p,j] (t=1)
    eye3 = np.eye(3, dtype=np.float32)
    w_in = np.einsum('pj,cd->jcpd', c_np, eye3).reshape(63, 630)
    w_tg = np.einsum('pj,cd->jcpd', -t_np, eye3).reshape(63, 630)
    return np.ascontiguousarray(np.concatenate([w_in, w_tg], axis=0))  # [126, 630]


_W = _build_w()

# ---------------------------------------------------------------- bass build
R_PER_GRP = 8                       # 128-row tiles per transpose group
N_GRP = B_CORE // (128 * R_PER_GRP)  # 8
N_CCH = 5                           # 630 = 5 x 126 output-column chunks
DMA_GRPS = 4                        # staging loads of 2048 rows each

_NC = None


def _build_bass(n_reps=1, timing_mode=False):
    import ml_dtypes
    import concourse.bacc as bacc
    import concourse.mybir as mybir
    import concourse.tile as tile

    f32 = mybir.dt.float32
    bf16 = mybir.dt.bfloat16
    nc = bacc.Bacc("TRN2", target_bir_lowering=False, debug=False)

    # timing_mode: x/y are Internal scratch (garbage data, identical
    # instruction stream) so benchmark dispatches ship no input bytes.
    kind = "Internal" if timing_mode else "ExternalInput"
    x = nc.dram_tensor("x", [B_CORE, 63], f32, kind=kind)
    y = nc.dram_tensor("y", [B_CORE, 63], f32, kind=kind)
    out = nc.dram_tensor("out", [126, 1], f32, kind="ExternalOutput")

    w_bf = _W.astype(ml_dtypes.bfloat16)
    assert np.array_equal(w_bf.astype(np.float32), _W)  # {-1,0,1} exact
    w_dram = nc.inline_tensor(w_bf, name="w_const")
    ident_dram = nc.inline_tensor(
        np.eye(128).astype(ml_dtypes.bfloat16), name="ident_const")

    d_rows = B_CORE // DMA_GRPS          # 2048 rows per staging load
    rsub = d_rows // 128                 # 16 row-chunks per staging tile
    rows = 128 * R_PER_GRP               # 1024 rows per transpose group

    with tile.TileContext(nc) as tc:
        with (
            tc.tile_pool(name="consts", bufs=1) as consts,
            tc.tile_pool(name="stx", bufs=2) as stx_pool,
            tc.tile_pool(name="sty", bufs=2) as sty_pool,
            tc.tile_pool(name="stb", bufs=3) as stb_pool,
            tc.tile_pool(name="zt", bufs=3) as zt_pool,
            tc.tile_pool(name="accp", bufs=2) as acc_pool,
            tc.tile_pool(name="psumT", bufs=2, space="PSUM") as psumT_pool,
            tc.tile_pool(name="psumD", bufs=3, space="PSUM") as psumD_pool,
            tc.tile_pool(name="misc", bufs=1) as misc,
        ):
            w_sb = consts.tile([126, 630], bf16)
            nc.sync.dma_start(w_sb[:], w_dram[:])
            id_sb = consts.tile([128, 128], bf16)
            nc.sync.dma_start(id_sb[:], ident_dram[:])

            scratch = misc.tile([126, 1024], f32)  # ACT abs dump (never read)

            # greedy ACT/DVE balance (errata-adjusted ns-per-op estimates:
            # ACT psum (172+FD)/1.2, DVE psum (120+FD)/0.96 at FD=1024)
            eng_t = {"act": 0.0, "dve": 0.0}

            def pick_engine(act_ns, dve_ns):
                e = "act" if eng_t["act"] + act_ns <= \
                    eng_t["dve"] + dve_ns else "dve"
                eng_t[e] += act_ns if e == "act" else dve_ns
                return e

            def emit_absred(col, dps):
                if pick_engine(997.0, 1192.0) == "act":
                    nc.scalar.activation(
                        scratch[:], dps[:],
                        mybir.ActivationFunctionType.Abs, accum_out=col)
                else:
                    nc.vector.tensor_reduce(
                        col, dps[:], axis=mybir.AxisListType.X,
                        op=mybir.AluOpType.add, apply_absolute_value=True)

            for rep in range(n_reps):
                acc = acc_pool.tile([126, N_GRP * N_CCH], f32, tag="acc")
                final = acc_pool.tile([126, 1], f32, tag="final")

                prev = None  # (zt, g) pending matmul+epilogue

                def flush_prev():
                    zt, g = prev
                    for c in range(N_CCH):
                        # two bf16 matmuls fill a 2-bank psum tile; one
                        # 1024-wide fused abs+sum drains it
                        dps = psumD_pool.tile([126, 1024], f32)
                        wc = w_sb[:, c * 126:(c + 1) * 126]
                        nc.tensor.matmul(
                            dps[:, 0:512], wc, zt[:, 0:512])
                        nc.tensor.matmul(
                            dps[:, 512:1024], wc, zt[:, 512:1024])
                        emit_absred(
                            acc[:, g * N_CCH + c: g * N_CCH + c + 1], dps)

                st_x = st_y = None
                for g in range(N_GRP):
                    if g % 2 == 0:
                        # contiguous staging: one 4032B run per partition
                        # per DMA (partition p <- 16 consecutive DRAM rows)
                        d = g // 2
                        st_x = stx_pool.tile([128, rsub, 63], f32)
                        st_y = sty_pool.tile([128, rsub, 63], f32)
                        xv = x[d * d_rows:(d + 1) * d_rows, :].rearrange(
                            "(p r) j -> p r j", p=128)
                        yv = y[d * d_rows:(d + 1) * d_rows, :].rearrange(
                            "(p r) j -> p r j", p=128)
                        nc.sync.dma_start(st_x[:, :, :], xv)
                        nc.sync.dma_start(st_y[:, :, :], yv)

                    # GPSIMD casts fp32->bf16 and interleaves x|y columns
                    h = (g % 2) * R_PER_GRP
                    stb = stb_pool.tile([128, R_PER_GRP, 126], bf16)
                    nc.gpsimd.tensor_copy(
                        stb[:, :, 0:63], st_x[:, h:h + R_PER_GRP, :])
                    nc.gpsimd.tensor_copy(
                        stb[:, :, 63:126], st_y[:, h:h + R_PER_GRP, :])

                    # transpose 8x [128,126] -> one [126,1024] bf16 psum tile
                    zt_ps = psumT_pool.tile([126, 1024], bf16)
                    for r in range(R_PER_GRP):
                        nc.tensor.transpose(
                            zt_ps[:, r * 128:(r + 1) * 128],
                            stb[:, r, :],
                            id_sb[:])

                    zt = zt_pool.tile([126, 1024], bf16)
                    # 16-bit psum: DVE copies in 2x_1P mode; keep them off
                    # ACT so ACT spends its cycles on drains
                    nc.vector.tensor_copy(zt[:], zt_ps[:])
                    eng_t["dve"] += 658.0

                    # software pipeline: matmuls for the PREVIOUS group run
                    # after this group's transposes, so the PE never waits
                    # on the psum->sbuf copy of its rhs
                    if prev is not None:
                        flush_prev()
                    prev = (zt, g)

                flush_prev()

                nc.vector.tensor_reduce(
                    final[:], acc[:], axis=mybir.AxisListType.X,
                    op=mybir.AluOpType.add)
                nc.sync.dma_start(out[:], final[:])

    nc.compile()
    return nc


def kernel(input, target):
    global _NC
    from concourse.bass_utils import run_bass_kernel_spmd

    if _NC is None:
        _NC = _build_bass()

    inp = np.ascontiguousarray(np.asarray(input, dtype=np.float32))
    tgt = np.ascontiguousarray(np.asarray(target, dtype=np.float32))
    assert inp.shape == (B_FULL, NJ * 3) and tgt.shape == (B_FULL, NJ * 3)

    in_maps = []
    for i in range(N_CORES):
        sl = slice(i * B_CORE, (i + 1) * B_CORE)
        in_maps.append({
            "x": np.ascontiguousarray(inp[sl]),
            "y": np.ascontiguousarray(tgt[sl]),
        })

    res = run_bass_kernel_spmd(_NC, in_maps, core_ids=list(range(N_CORES)))
    total = np.float64(0.0)
    for r in res.results:
        total += np.float64(r["out"].astype(np.float64).sum())
    return np.array([total / B_FULL], dtype=np.float32)


# revision 15
# speedup vs baseline: 691.7079x; 1.2146x over previous
"""Trainium2 Bass kernel for the compositional skeleton loss.

loss = mean_b sum_{pairs p, xyz c} | (C @ bones_in)[b,p,c] - (T @ bones_tgt)[b,p,c] |

Reformulated as one matmul per batch row:  delta_row = z_row @ W, where
z_row = [input_row (63), target_row (63)] and W is [126, 630] built from the
signed path-sum matrix C and the endpoint-diff matrix T (block structure over
the 3 xyz channels), followed by abs + total sum, / B.

v5 pipeline (per 8192-row core shard):
  DMA      x,y -> separate contiguous staging tiles (1 descriptor/partition;
           the old interleaved layout fragmented every DMA into 252B runs and
           made the kernel DMA-descriptor-bound at ~32us)
  GPSIMD   fp32 -> bf16 cast, interleaving x|y into [128, 8, 126] tiles
           (strided engine writes are free; DMA strides are not)
  PE       8x [128,126] transposes per 1024-row group (bf16, 1 cyc/row,
           1-bank psum) + 10 bf16 matmuls vs W chunks (1 cyc/row)
  DVE      psum->sbuf copies of z^T (2x mode on 16-bit)
  ACT/DVE  fused abs+accumulate drains of the [126,1024] fp32 psum tiles,
           greedy-balanced by measured per-op cost
Sharding: pure data parallel over batch across 8 NeuronCores; host sums the
eight [126,1] partial outputs.
"""

import numpy as np
from collections import deque
from itertools import combinations

# ---------------------------------------------------------------- constants
NJ = 21
B_FULL = 65536
N_CORES = 8
B_CORE = B_FULL // N_CORES  # 8192

_JOINTS = ['Ab', 'Chest', 'Head', 'Hip', 'LFArm', 'LFoot', 'LHand', 'LShin',
           'LShoulder', 'LThigh', 'LToe', 'LUArm', 'Neck', 'RFArm', 'RFoot',
           'RHand', 'RShin', 'RShoulder', 'RThigh', 'RToe', 'RUArm']
_PARENTS = {'Ab': 'Hip', 'Chest': 'Ab', 'Head': 'Neck', 'Hip': 'Hip',
            'LFArm': 'LUArm', 'LFoot': 'LShin', 'LHand': 'LFArm',
            'LShin': 'LThigh', 'LShoulder': 'Chest', 'LThigh': 'Hip',
            'LToe': 'LFoot', 'LUArm': 'LShoulder', 'Neck': 'Chest',
            'RFArm': 'RUArm', 'RFoot': 'RShin', 'RHand': 'RFArm',
            'RShin': 'RThigh', 'RShoulder': 'Chest', 'RThigh': 'Hip',
            'RToe': 'RFoot', 'RUArm': 'RShoulder'}


def _build_w():
    idx = {n: i for i, n in enumerate(_JOINTS)}
    par = {idx[k]: idx[v] for k, v in _PARENTS.items()}
    adj = {j: [] for j in range(NJ)}
    for j, p in par.items():
        if j != p:
            adj[j].append(p)
            adj[p].append(j)

    def bfs_path(u, v):
        prev = {u: None}
        q = deque([u])
        while q:
            x = q.popleft()
            if x == v:
                break
            for y in adj[x]:
                if y not in prev:
                    prev[y] = x
                    q.append(y)
        path = [v]
        while prev[path[-1]] is not None:
            path.append(prev[path[-1]])
        return path[::-1]

    pairs = list(combinations(range(NJ), 2))  # 210
    c_np = np.zeros((len(pairs), NJ), np.float32)
    t_np = np.zeros((len(pairs), NJ), np.float32)
    for pi, (u, v) in enumerate(pairs):
        pa = bfs_path(u, v)
        for m in range(len(pa) - 1):
            c_np[pi, pa[m]] += 1.0 if par[pa[m]] == pa[m + 1] else -1.0
        t_np[pi, u] += 1.0
        t_np[pi, v] -= 1.0

    # W[t*63 + j*3 + c, p*3 + c] = C[p,j] (t=0) / -T[p,j] (t=1)
    eye3 = np.eye(3, dtype=np.float32)
    w_in = np.einsum('pj,cd->jcpd', c_np, eye3).reshape(63, 630)
    w_tg = np.einsum('pj,cd->jcpd', -t_np, eye3).reshape(63, 630)
    return np.ascontiguousarray(np.concatenate([w_in, w_tg], axis=0))  # [126, 630]


_W = _build_w()

# ---------------------------------------------------------------- bass build
R_PER_GRP = 8                       # 128-row tiles per transpose group
N_GRP = B_CORE // (128 * R_PER_GRP)  # 8
N_CCH = 5                           # 630 = 5 x 126 output-column chunks
DMA_GRPS = 2                        # staging loads of 4096 rows: 8KB/partition
                                    # runs halve DMA descriptor count (the
                                    # ~184ns/descriptor overhead dominates)

_NC = None


def _build_bass(n_reps=1, timing_mode=False, drain_inplace=True,
                act_pref=-4000.0):
    # drain_inplace: ACT abs-drains write their (unused) main output back
    # into the psum tile being read — the psum port is lower-latency than
    # SBUF for ScalarE (measured -1.3us/rep vs an SBUF scratch dump).
    # act_pref biases the greedy ACT/DVE split ACT-ward: hardware-scanned
    # optimum (-3500..-5000ns ~= 4 extra drains on ACT vs the errata-model
    # balance; ACT's real psum-drain rate beats the model relative to DVE).
    import ml_dtypes
    import concourse.bacc as bacc
    import concourse.mybir as mybir
    import concourse.tile as tile

    f32 = mybir.dt.float32
    bf16 = mybir.dt.bfloat16
    nc = bacc.Bacc("TRN2", target_bir_lowering=False, debug=False)

    # timing_mode: x/y are Internal scratch (garbage data, identical
    # instruction stream) so benchmark dispatches ship no input bytes.
    kind = "Internal" if timing_mode else "ExternalInput"
    x = nc.dram_tensor("x", [B_CORE, 63], f32, kind=kind)
    y = nc.dram_tensor("y", [B_CORE, 63], f32, kind=kind)
    out = nc.dram_tensor("out", [126, 1], f32, kind="ExternalOutput")

    w_bf = _W.astype(ml_dtypes.bfloat16)
    assert np.array_equal(w_bf.astype(np.float32), _W)  # {-1,0,1} exact
    w_dram = nc.inline_tensor(w_bf, name="w_const")
    ident_dram = nc.inline_tensor(
        np.eye(128).astype(ml_dtypes.bfloat16), name="ident_const")

    d_rows = B_CORE // DMA_GRPS          # 2048 rows per staging load
    rsub = d_rows // 128                 # 16 row-chunks per staging tile
    rows = 128 * R_PER_GRP               # 1024 rows per transpose group

    with tile.TileContext(nc) as tc:
        with (
            tc.tile_pool(name="consts", bufs=1) as consts,
            tc.tile_pool(name="stx", bufs=2) as stx_pool,
            tc.tile_pool(name="sty", bufs=2) as sty_pool,
            tc.tile_pool(name="stb", bufs=3) as stb_pool,
            tc.tile_pool(name="zt", bufs=3) as zt_pool,
            tc.tile_pool(name="accp", bufs=2) as acc_pool,
            tc.tile_pool(name="psumT", bufs=2, space="PSUM") as psumT_pool,
            tc.tile_pool(name="psumD", bufs=3, space="PSUM") as psumD_pool,
            tc.tile_pool(name="misc", bufs=1) as misc,
        ):
            w_sb = consts.tile([126, 630], bf16)
            nc.sync.dma_start(w_sb[:], w_dram[:])
            id_sb = consts.tile([128, 128], bf16)
            nc.sync.dma_start(id_sb[:], ident_dram[:])

            scratch = misc.tile([126, 1024], f32)  # ACT abs dump (never read)

            # greedy ACT/DVE balance (errata-adjusted ns-per-op estimates:
            # ACT psum (172+FD)/1.2, DVE psum (120+FD)/0.96 at FD=1024)
            eng_t = {"act": act_pref, "dve": 0.0}

            def pick_engine(act_ns, dve_ns):
                e = "act" if eng_t["act"] + act_ns <= \
                    eng_t["dve"] + dve_ns else "dve"
                eng_t[e] += act_ns if e == "act" else dve_ns
                return e

            def emit_absred(col, dps):
                if pick_engine(997.0, 1192.0) == "act":
                    main_out = dps[:] if drain_inplace else scratch[:]
                    nc.scalar.activation(
                        main_out, dps[:],
                        mybir.ActivationFunctionType.Abs, accum_out=col)
                else:
                    nc.vector.tensor_reduce(
                        col, dps[:], axis=mybir.AxisListType.X,
                        op=mybir.AluOpType.add, apply_absolute_value=True)

            for rep in range(n_reps):
                acc = acc_pool.tile([126, N_GRP * N_CCH], f32, tag="acc")
                final = acc_pool.tile([126, 1], f32, tag="final")

                prev = None  # (zt, g) pending matmul+epilogue

                def flush_prev():
                    zt, g = prev
                    for c in range(N_CCH):
                        # two bf16 matmuls fill a 2-bank psum tile; one
                        # 1024-wide fused abs+sum drains it
                        dps = psumD_pool.tile([126, 1024], f32)
                        wc = w_sb[:, c * 126:(c + 1) * 126]
                        nc.tensor.matmul(
                            dps[:, 0:512], wc, zt[:, 0:512])
                        nc.tensor.matmul(
                            dps[:, 512:1024], wc, zt[:, 512:1024])
                        emit_absred(
                            acc[:, g * N_CCH + c: g * N_CCH + c + 1], dps)

                st_x = st_y = None
                grps_per_load = N_GRP // DMA_GRPS
                for g in range(N_GRP):
                    if g % grps_per_load == 0:
                        # contiguous staging: one run per partition per DMA
                        # (partition p <- rsub consecutive DRAM rows)
                        d = g // grps_per_load
                        st_x = stx_pool.tile([128, rsub, 63], f32)
                        st_y = sty_pool.tile([128, rsub, 63], f32)
                        xv = x[d * d_rows:(d + 1) * d_rows, :].rearrange(
                            "(p r) j -> p r j", p=128)
                        yv = y[d * d_rows:(d + 1) * d_rows, :].rearrange(
                            "(p r) j -> p r j", p=128)
                        nc.sync.dma_start(st_x[:, :, :], xv)
                        nc.sync.dma_start(st_y[:, :, :], yv)

                    # GPSIMD casts fp32->bf16 and interleaves x|y columns
                    h = (g % grps_per_load) * R_PER_GRP
                    stb = stb_pool.tile([128, R_PER_GRP, 126], bf16)
                    nc.gpsimd.tensor_copy(
                        stb[:, :, 0:63], st_x[:, h:h + R_PER_GRP, :])
                    nc.gpsimd.tensor_copy(
                        stb[:, :, 63:126], st_y[:, h:h + R_PER_GRP, :])

                    # transpose 8x [128,126] -> one [126,1024] bf16 psum tile
                    zt_ps = psumT_pool.tile([126, 1024], bf16)
                    for r in range(R_PER_GRP):
                        nc.tensor.transpose(
                            zt_ps[:, r * 128:(r + 1) * 128],
                            stb[:, r, :],
                            id_sb[:])

                    zt = zt_pool.tile([126, 1024], bf16)
                    # 16-bit psum: DVE copies in 2x_1P mode; keep them off
                    # ACT so ACT spends its cycles on drains
                    nc.vector.tensor_copy(zt[:], zt_ps[:])
                    eng_t["dve"] += 658.0

                    # software pipeline: matmuls for the PREVIOUS group run
                    # after this group's transposes, so the PE never waits
                    # on the psum->sbuf copy of its rhs
                    if prev is not None:
                        flush_prev()
                    prev = (zt, g)

                flush_prev()

                nc.vector.tensor_reduce(
                    final[:], acc[:], axis=mybir.AxisListType.X,
                    op=mybir.AluOpType.add)
                nc.sync.dma_start(out[:], final[:])

    nc.compile()
    return nc


def kernel(input, target):
    global _NC
    from concourse.bass_utils import run_bass_kernel_spmd

    if _NC is None:
        _NC = _build_bass()

    inp = np.ascontiguousarray(np.asarray(input, dtype=np.float32))
    tgt = np.ascontiguousarray(np.asarray(target, dtype=np.float32))
    assert inp.shape == (B_FULL, NJ * 3) and tgt.shape == (B_FULL, NJ * 3)

    in_maps = []
    for i in range(N_CORES):
        sl = slice(i * B_CORE, (i + 1) * B_CORE)
        in_maps.append({
            "x": np.ascontiguousarray(inp[sl]),
            "y": np.ascontiguousarray(tgt[sl]),
        })

    res = run_bass_kernel_spmd(_NC, in_maps, core_ids=list(range(N_CORES)))
    total = np.float64(0.0)
    for r in res.results:
        total += np.float64(r["out"].astype(np.float64).sum())
    return np.array([total / B_FULL], dtype=np.float32)


# revision 19
# speedup vs baseline: 697.0134x; 1.0077x over previous
"""Trainium2 Bass kernel for the compositional skeleton loss.

loss = mean_b sum_{pairs p, xyz c} | (C @ bones_in)[b,p,c] - (T @ bones_tgt)[b,p,c] |

Reformulated as one matmul per batch row:  delta_row = z_row @ W, where
z_row = [input_row (63), target_row (63)] and W is [126, 630] built from the
signed path-sum matrix C and the endpoint-diff matrix T (block structure over
the 3 xyz channels), followed by abs + total sum, / B.

v5 pipeline (per 8192-row core shard):
  DMA      x,y -> separate contiguous staging tiles (1 descriptor/partition;
           the old interleaved layout fragmented every DMA into 252B runs and
           made the kernel DMA-descriptor-bound at ~32us)
  GPSIMD   fp32 -> bf16 cast, interleaving x|y into [128, 8, 126] tiles
           (strided engine writes are free; DMA strides are not)
  PE       8x [128,126] transposes per 1024-row group (bf16, 1 cyc/row,
           1-bank psum) + 10 bf16 matmuls vs W chunks (1 cyc/row)
  DVE      psum->sbuf copies of z^T (2x mode on 16-bit)
  ACT/DVE  fused abs+accumulate drains of the [126,1024] fp32 psum tiles,
           greedy-balanced by measured per-op cost
Sharding: pure data parallel over batch across 8 NeuronCores; host sums the
eight [126,1] partial outputs.
"""

import numpy as np
from collections import deque
from itertools import combinations

# ---------------------------------------------------------------- constants
NJ = 21
B_FULL = 65536
N_CORES = 8
B_CORE = B_FULL // N_CORES  # 8192

_JOINTS = ['Ab', 'Chest', 'Head', 'Hip', 'LFArm', 'LFoot', 'LHand', 'LShin',
           'LShoulder', 'LThigh', 'LToe', 'LUArm', 'Neck', 'RFArm', 'RFoot',
           'RHand', 'RShin', 'RShoulder', 'RThigh', 'RToe', 'RUArm']
_PARENTS = {'Ab': 'Hip', 'Chest': 'Ab', 'Head': 'Neck', 'Hip': 'Hip',
            'LFArm': 'LUArm', 'LFoot': 'LShin', 'LHand': 'LFArm',
            'LShin': 'LThigh', 'LShoulder': 'Chest', 'LThigh': 'Hip',
            'LToe': 'LFoot', 'LUArm': 'LShoulder', 'Neck': 'Chest',
            'RFArm': 'RUArm', 'RFoot': 'RShin', 'RHand': 'RFArm',
            'RShin': 'RThigh', 'RShoulder': 'Chest', 'RThigh': 'Hip',
            'RToe': 'RFoot', 'RUArm': 'RShoulder'}


def _build_w():
    idx = {n: i for i, n in enumerate(_JOINTS)}
    par = {idx[k]: idx[v] for k, v in _PARENTS.items()}
    adj = {j: [] for j in range(NJ)}
    for j, p in par.items():
        if j != p:
            adj[j].append(p)
            adj[p].append(j)

    def bfs_path(u, v):
        prev = {u: None}
        q = deque([u])
        while q:
            x = q.popleft()
            if x == v:
                break
            for y in adj[x]:
                if y not in prev:
                    prev[y] = x
                    q.append(y)
        path = [v]
        while prev[path[-1]] is not None:
            path.append(prev[path[-1]])
        return path[::-1]

    pairs = list(combinations(range(NJ), 2))  # 210
    c_np = np.zeros((len(pairs), NJ), np.float32)
    t_np = np.zeros((len(pairs), NJ), np.float32)
    for pi, (u, v) in enumerate(pairs):
        pa = bfs_path(u, v)
        for m in range(len(pa) - 1):
            c_np[pi, pa[m]] += 1.0 if par[pa[m]] == pa[m + 1] else -1.0
        t_np[pi, u] += 1.0
        t_np[pi, v] -= 1.0

    # W[t*63 + j*3 + c, p*3 + c] = C[p,j] (t=0) / -T[p,j] (t=1)
    eye3 = np.eye(3, dtype=np.float32)
    w_in = np.einsum('pj,cd->jcpd', c_np, eye3).reshape(63, 630)
    w_tg = np.einsum('pj,cd->jcpd', -t_np, eye3).reshape(63, 630)
    return np.ascontiguousarray(np.concatenate([w_in, w_tg], axis=0))  # [126, 630]


_W = _build_w()

# ---------------------------------------------------------------- bass build
R_PER_GRP = 8                       # 128-row tiles per transpose group
N_GRP = B_CORE // (128 * R_PER_GRP)  # 8
N_CCH = 5                           # 630 = 5 x 126 output-column chunks
DMA_GRPS = 2                        # staging loads of 4096 rows: 8KB/partition
                                    # runs halve DMA descriptor count (the
                                    # ~184ns/descriptor overhead dominates)

_NC = None


def _build_bass(n_reps=1, timing_mode=False, drain_inplace=True,
                act_pref=-4000.0, use_xbar=False):
    # drain_inplace: ACT abs-drains write their (unused) main output back
    # into the psum tile being read — the psum port is lower-latency than
    # SBUF for ScalarE (measured -1.3us/rep vs an SBUF scratch dump).
    # act_pref biases the greedy ACT/DVE split ACT-ward: hardware-scanned
    # optimum (-3500..-5000ns ~= 4 extra drains on ACT vs the errata-model
    # balance; ACT's real psum-drain rate beats the model relative to DVE).
    import ml_dtypes
    import concourse.bacc as bacc
    import concourse.mybir as mybir
    import concourse.tile as tile

    f32 = mybir.dt.float32
    bf16 = mybir.dt.bfloat16
    nc = bacc.Bacc("TRN2", target_bir_lowering=False, debug=False)

    # timing_mode: x/y are Internal scratch (garbage data, identical
    # instruction stream) so benchmark dispatches ship no input bytes.
    kind = "Internal" if timing_mode else "ExternalInput"
    x = nc.dram_tensor("x", [B_CORE, 63], f32, kind=kind)
    y = nc.dram_tensor("y", [B_CORE, 63], f32, kind=kind)
    out = nc.dram_tensor("out", [126, 1], f32, kind="ExternalOutput")

    w_bf = _W.astype(ml_dtypes.bfloat16)
    assert np.array_equal(w_bf.astype(np.float32), _W)  # {-1,0,1} exact
    w_dram = nc.inline_tensor(w_bf, name="w_const")
    ident_dram = nc.inline_tensor(
        np.eye(128).astype(ml_dtypes.bfloat16), name="ident_const")

    d_rows = B_CORE // DMA_GRPS          # 2048 rows per staging load
    rsub = d_rows // 128                 # 16 row-chunks per staging tile
    rows = 128 * R_PER_GRP               # 1024 rows per transpose group

    with tile.TileContext(nc) as tc:
        with (
            tc.tile_pool(name="consts", bufs=1) as consts,
            tc.tile_pool(name="stx", bufs=2) as stx_pool,
            tc.tile_pool(name="sty", bufs=2) as sty_pool,
            tc.tile_pool(name="stb", bufs=3) as stb_pool,
            tc.tile_pool(name="zt", bufs=3) as zt_pool,
            tc.tile_pool(name="accp", bufs=2) as acc_pool,
            tc.tile_pool(name="psumT", bufs=(1 if use_xbar else 2),
                         space="PSUM") as psumT_pool,
            tc.tile_pool(name="psumD", bufs=(4 if use_xbar else 3),
                         space="PSUM") as psumD_pool,
            tc.tile_pool(name="misc", bufs=1) as misc,
        ):
            w_sb = consts.tile([126, 630], bf16)
            nc.sync.dma_start(w_sb[:], w_dram[:])
            id_sb = consts.tile([128, 128], bf16)
            nc.sync.dma_start(id_sb[:], ident_dram[:])

            scratch = misc.tile([126, 1024], f32)  # ACT abs dump (never read)

            # greedy ACT/DVE balance (errata-adjusted ns-per-op estimates:
            # ACT psum (172+FD)/1.2, DVE psum (120+FD)/0.96 at FD=1024)
            eng_t = {"act": act_pref, "dve": 0.0}

            def pick_engine(act_ns, dve_ns):
                e = "act" if eng_t["act"] + act_ns <= \
                    eng_t["dve"] + dve_ns else "dve"
                eng_t[e] += act_ns if e == "act" else dve_ns
                return e

            def emit_absred(col, dps):
                if pick_engine(997.0, 1192.0) == "act":
                    main_out = dps[:] if drain_inplace else scratch[:]
                    nc.scalar.activation(
                        main_out, dps[:],
                        mybir.ActivationFunctionType.Abs, accum_out=col)
                else:
                    nc.vector.tensor_reduce(
                        col, dps[:], axis=mybir.AxisListType.X,
                        op=mybir.AluOpType.add, apply_absolute_value=True)

            for rep in range(n_reps):
                acc = acc_pool.tile([126, N_GRP * N_CCH], f32, tag="acc")
                final = acc_pool.tile([126, 1], f32, tag="final")

                prev = None  # (zt, g) pending matmul+epilogue

                def flush_prev():
                    zt, g = prev
                    for c in range(N_CCH):
                        # two bf16 matmuls fill a 2-bank psum tile; one
                        # 1024-wide fused abs+sum drains it
                        dps = psumD_pool.tile([126, 1024], f32)
                        wc = w_sb[:, c * 126:(c + 1) * 126]
                        nc.tensor.matmul(
                            dps[:, 0:512], wc, zt[0:126, 0:512])
                        nc.tensor.matmul(
                            dps[:, 512:1024], wc, zt[0:126, 512:1024])
                        emit_absred(
                            acc[:, g * N_CCH + c: g * N_CCH + c + 1], dps)

                st_x = st_y = None
                grps_per_load = N_GRP // DMA_GRPS
                for g in range(N_GRP):
                    if g % grps_per_load == 0:
                        # contiguous staging: one run per partition per DMA
                        # (partition p <- rsub consecutive DRAM rows)
                        d = g // grps_per_load
                        st_x = stx_pool.tile([128, rsub, 63], f32)
                        st_y = sty_pool.tile([128, rsub, 63], f32)
                        xv = x[d * d_rows:(d + 1) * d_rows, :].rearrange(
                            "(p r) j -> p r j", p=128)
                        yv = y[d * d_rows:(d + 1) * d_rows, :].rearrange(
                            "(p r) j -> p r j", p=128)
                        nc.sync.dma_start(st_x[:, :, :], xv)
                        nc.sync.dma_start(st_y[:, :, :], yv)

                    # GPSIMD casts fp32->bf16 and interleaves x|y columns
                    h = (g % grps_per_load) * R_PER_GRP
                    cols = 128 if use_xbar else 126
                    stb = stb_pool.tile([128, R_PER_GRP, cols], bf16)
                    nc.gpsimd.tensor_copy(
                        stb[:, :, 0:63], st_x[:, h:h + R_PER_GRP, :])
                    nc.gpsimd.tensor_copy(
                        stb[:, :, 63:126], st_y[:, h:h + R_PER_GRP, :])

                    if use_xbar:
                        # SBUF->SBUF DMA crossbar transpose (16-bit only):
                        # no PE transposes, no psum staging, no ACT/DVE
                        # copies.  Rows 126-127 of zt are pad, never read.
                        zt = zt_pool.tile([128, 1024], bf16)
                        for r in range(R_PER_GRP):
                            nc.sync.dma_start(
                                zt[:, r * 128:(r + 1) * 128],
                                stb[:, r, :], transpose=True)
                    else:
                        # transpose 8x [128,126] -> [126,1024] bf16 psum tile
                        zt_ps = psumT_pool.tile([126, 1024], bf16)
                        for r in range(R_PER_GRP):
                            nc.tensor.transpose(
                                zt_ps[:, r * 128:(r + 1) * 128],
                                stb[:, r, :],
                                id_sb[:])
                        zt = zt_pool.tile([126, 1024], bf16)
                        # 16-bit psum: DVE copies in 2x_1P mode; keep them
                        # off ACT so ACT spends its cycles on drains
                        nc.vector.tensor_copy(zt[:], zt_ps[:])
                        eng_t["dve"] += 658.0

                    # software pipeline: matmuls for the PREVIOUS group run
                    # after this group's transposes, so the PE never waits
                    # on the psum->sbuf copy of its rhs
                    if prev is not None:
                        flush_prev()
                    prev = (zt, g)

                flush_prev()

                nc.vector.tensor_reduce(
                    final[:], acc[:], axis=mybir.AxisListType.X,
                    op=mybir.AluOpType.add)
                nc.sync.dma_start(out[:], final[:])

    nc.compile()
    return nc


def kernel(input, target):
    global _NC
    from concourse.bass_utils import run_bass_kernel_spmd

    if _NC is None:
        _NC = _build_bass()

    inp = np.ascontiguousarray(np.asarray(input, dtype=np.float32))
    tgt = np.ascontiguousarray(np.asarray(target, dtype=np.float32))
    assert inp.shape == (B_FULL, NJ * 3) and tgt.shape == (B_FULL, NJ * 3)

    in_maps = []
    for i in range(N_CORES):
        sl = slice(i * B_CORE, (i + 1) * B_CORE)
        in_maps.append({
            "x": np.ascontiguousarray(inp[sl]),
            "y": np.ascontiguousarray(tgt[sl]),
        })

    res = run_bass_kernel_spmd(_NC, in_maps, core_ids=list(range(N_CORES)))
    total = np.float64(0.0)
    for r in res.results:
        total += np.float64(r["out"].astype(np.float64).sum())
    return np.array([total / B_FULL], dtype=np.float32)
